# revision 51
# baseline (speedup 1.0000x reference)
"""AttentionBlock (GroupNorm + MHSA + proj + residual) on 8 TRN2 NeuronCores.

Sharding: data-parallel over batch (B=8 -> 1 batch element per core), SPMD.

Software-pipelined attention (352us baseline -> ~154us). The
scores->exp->av dependency chain is decoupled by keeping each pair's
exp(S) tiles (pt) in SBUF: pair p's slots run scores(p) + av(p-1) +
qk-projection(p+1) back to back on the PE, with exp (ScalarE exact for
even heads / VectorE Schraudolph bf16 fast-exp for odd heads) hidden
underneath; scores for the two heads of a pair run concurrently on row
groups 0/64 of the PE array. PSUM banks: scores 2x2, av accumulators 2x1,
qk/zb 1x2. Softmax 1/Z is computed as exp(-ln Z) on ScalarE in three
batches so the last heads' normalization hides under the tail's av; the
out-projection is woven into the tail as each normalized a-tile lands.
Other key points vs the original baseline: bf16 weights/activations for
everything except the fp32 GroupNorm + residual, x-first DMA ordering,
PE warm-up + filler matmuls so the HAM clock gate holds 2.4GHz through
low-density phases, GroupNorm rstd via exp(-0.5*ln(var+eps)) so only one
ACT table set is ever loaded, and biases folded into ScalarE psum->sbuf
copies / the residual instead of matmuls.
"""

import sys
import numpy as np

sys.path.insert(0, "/opt/trn_rl_repo")

import concourse.bacc as bacc
import concourse.bass as bass
import concourse.mybir as mybir
import concourse.tile as tile
from concourse import bass_utils

F32 = mybir.dt.float32
F32R = mybir.dt.float32r
BF16 = mybir.dt.bfloat16
I16 = mybir.dt.int16
AF = mybir.ActivationFunctionType
ALU = mybir.AluOpType

B, C, HH, WW = 8, 512, 32, 32
T = HH * WW            # 1024
NH = 8                 # heads
CH = C // NH           # 64 per-head dim
NCT = C // 128         # 4 channel tiles
NTT = T // 128         # 8 seq tiles
SCALE = 1.0 / np.sqrt(np.sqrt(CH))
EPS = 1e-5
VW = NH * (CH + 1)     # 520: v-section width incl per-head Z column

# Schraudolph fast-exp constants, bf16 flavor: i16 = rint(S * EA + EB)
# gives the int16 bit pattern of bf16(~exp(S)). Max multiplicative error
# ~ +-3.5% (centered); cancels partially in softmax.
EA = float((1 << 7) / np.log(2.0))
EB = float(127.0 * (1 << 7) - 0.5 * 0.0860713320559342 * (1 << 7) + 0.5)

_CACHE = {}


def build_kernel(debug=False):
    nc = bacc.Bacc(
        "TRN2", target_bir_lowering=False, debug=debug, num_devices=8
    )

    x_d = nc.dram_tensor("x", (C, T), F32, kind="ExternalInput")
    wqkvT_d = nc.dram_tensor("wqkvT", (C, 2 * C + VW), BF16, kind="ExternalInput")
    wprojT_d = nc.dram_tensor("wprojT", (C, C), BF16, kind="ExternalInput")
    cblob_d = nc.dram_tensor("cblob", (128, 2 * NCT + 8), F32, kind="ExternalInput")
    bcols_d = nc.dram_tensor("bcols", (128, 12), F32, kind="ExternalInput")
    vrow_d = nc.dram_tensor("vrow", (1, VW + 512), BF16, kind="ExternalInput")
    gbcast_d = nc.dram_tensor("gbcast", (8, 128), F32, kind="ExternalInput")
    e8_d = nc.dram_tensor("e8", (4, NH * CH), F32R, kind="ExternalInput")
    e2_d = nc.dram_tensor("e2", (2, NH * CH), F32R, kind="ExternalInput")
    y_d = nc.dram_tensor("y", (C, T), F32, kind="ExternalOutput")

    with tile.TileContext(nc) as tc:
        with (
            tc.tile_pool(name="single", bufs=1) as single,
            tc.tile_pool(name="hp", bufs=NCT) as hp,
            tc.tile_pool(name="vt", bufs=NTT) as vtp,
            tc.tile_pool(name="qk", bufs=4) as qkp,
            tc.tile_pool(name="ptA", bufs=2 * NTT) as ptAp,
            tc.tile_pool(name="ptB", bufs=2 * NTT) as ptBp,
            tc.tile_pool(name="aun", bufs=1) as aunp,
            tc.tile_pool(name="aall", bufs=NCT) as aallp,
            tc.tile_pool(name="zp", bufs=1) as zp,
            tc.tile_pool(name="tmp", bufs=2) as tmpp,
            tc.tile_pool(name="gn", bufs=8) as gnp,
            tc.tile_pool(name="ps", bufs=1, space="PSUM") as pp,
        ):
            # PSUM tags: sc 2x[128,1024] (4 banks), av 2x[128,512] (2),
            # qk 2x[128,512] (2) -> 8 banks exactly. av and qk rings are
            # 2-deep so no group or chunk ever waits on its predecessor's
            # psum->sbuf copy.
            def sc_tile(name):
                return pp.tile([128, T], F32, tag="sc", bufs=2, name=name)

            def av_tile(name):
                return pp.tile([128, 512], F32, tag="av", bufs=2, name=name)

            def qk_tile(name):
                return pp.tile([128, 512], F32, tag="qk", bufs=2, name=name)

            # ---------------- x first, then consts, then weights ----------
            xbig = single.tile([128, NCT, T], F32, tag="xbig")
            xr4 = x_d.ap().rearrange("(c p) t -> p c t", p=128)
            # half-tile granularity: each bn_stats chunk unblocks as soon as
            # its own 512 columns land
            for ct in range(NCT):
                for hf in range(2):
                    sl = slice(hf * 512, (hf + 1) * 512)
                    nc.sync.dma_start(out=xbig[:, ct, sl], in_=xr4[:, ct, sl])
            x_t = [xbig[:, ct, :] for ct in range(NCT)]

            cblob = single.tile([128, 2 * NCT + 8], F32, tag="cblob")
            nc.sync.dma_start(out=cblob[:, :], in_=cblob_d.ap())
            gamma = cblob[:, 0:NCT]
            beta = cblob[:, NCT:2 * NCT]
            gred = cblob[:, 2 * NCT:2 * NCT + 8]
            bcols = single.tile([128, 12], F32, tag="bcols")
            nc.sync.dma_start(out=bcols[:, :], in_=bcols_d.ap())
            gbcast = single.tile([8, 128], F32, tag="gbcast")
            nc.sync.dma_start(out=gbcast[:, :], in_=gbcast_d.ap())
            e8 = single.tile([4, NH * CH], F32R, tag="e8")
            nc.sync.dma_start(out=e8[:, :], in_=e8_d.ap())
            e2 = single.tile([2, NH * CH], F32R, tag="e2")
            nc.sync.dma_start(out=e2[:, :], in_=e2_d.ap())
            vrow = single.tile([1, VW + 512], BF16, tag="vrow")
            nc.sync.dma_start(out=vrow[:, :], in_=vrow_d.ap())
            vbias = vrow[:, 0:VW]
            ones = vrow[:, VW:VW + 512]

            # preload the natural_log_exp ACT table set during DMA wait
            tldm = gnp.tile([1, 1], F32, tag="tld")
            nc.scalar.activation(out=tldm[:, :], in_=cblob[0:1, 0:1], func=AF.Ln)

            wqbig = single.tile([128, NCT, 2 * C + VW], BF16, tag="wqbig")
            wqr = wqkvT_d.ap().rearrange("(c p) t -> p c t", p=128)
            # v-section first (vT is computed first), then qk columns
            nc.sync.dma_start(
                out=wqbig[:, :, 2 * C:2 * C + VW], in_=wqr[:, :, 2 * C:2 * C + VW]
            )
            nc.sync.dma_start(out=wqbig[:, :, 0:2 * C], in_=wqr[:, :, 0:2 * C])
            wq_t = [wqbig[:, ct, :] for ct in range(NCT)]
            wpbig = single.tile([128, NCT, C], BF16, tag="wpbig")
            nc.sync.dma_start(
                out=wpbig[:, :, :],
                in_=wprojT_d.ap().rearrange("(c p) t -> p c t", p=128),
            )
            wp_t = [wpbig[:, ct, :] for ct in range(NCT)]

            # ---------------- PE warm-up (HAM clock gate) -----------------
            # ~5us of fp32 matmuls on x tile 0 flip the PE clock gate to
            # 2.4GHz before the real matmul stream begins.
            def emit_warmups(lo, hi):
                for w in range(lo, hi):
                    wps = av_tile(f"warm{w}")
                    nc.tensor.matmul(
                        wps[:, :], x_t[0][:, 0:128], x_t[0][:, 0:512],
                        start=True, stop=True,
                    )

            emit_warmups(0, 6)

            # ---------------- GroupNorm ----------------
            # raw per-channel sums of x (VectorE reduce) and x^2 (ScalarE
            # Square with free-dim accumulator) in parallel; the 1/(16*1024)
            # normalization is folded into the host-side gred constant
            cs = gnp.tile([128, 2 * NCT], F32, tag="cs")
            sqscr = gnp.tile([128, T], F32, tag="sqscr", bufs=1)
            for ct in range(NCT):
                nc.vector.tensor_reduce(
                    out=cs[:, ct:ct + 1], in_=x_t[ct][:, :],
                    axis=mybir.AxisListType.X, op=ALU.add,
                )
                nc.scalar.activation(
                    out=sqscr[:, :], in_=x_t[ct][:, :], func=AF.Square,
                    accum_out=cs[:, NCT + ct:NCT + ct + 1],
                )
            gsp = av_tile("gsp")
            nc.tensor.matmul(
                gsp[0:8, 0:2 * NCT], gred[:, :], cs[:, :], start=True, stop=True
            )
            gs = gnp.tile([8, 2 * NCT], F32, tag="gs")
            nc.vector.tensor_copy(out=gs[:, :], in_=gsp[0:8, 0:2 * NCT])
            emit_warmups(6, 9)
            # var = E[x^2] - mean^2; rstd = exp(-0.5*ln(var+eps))
            t1 = gnp.tile([8, NCT], F32, tag="t1")
            veps = gnp.tile([8, NCT], F32, tag="veps")
            nc.vector.tensor_mul(out=t1[:, :], in0=gs[:, 0:NCT], in1=gs[:, 0:NCT])
            nc.vector.tensor_sub(out=veps[:, :], in0=gs[:, NCT:], in1=t1[:, :])
            nc.vector.tensor_scalar_add(out=veps[:, :], in0=veps[:, :], scalar1=EPS)
            lv = gnp.tile([8, NCT], F32, tag="lv")
            nc.scalar.activation(out=lv[:, :], in_=veps[:, :], func=AF.Ln)
            r0 = gnp.tile([8, NCT], F32, tag="r0")
            nc.scalar.activation(out=r0[:, :], in_=lv[:, :], func=AF.Exp, scale=-0.5)
            # de-interleaved [means | rstds] layout: one broadcast matmul
            # and three batched DVE ops replace 4 matmuls + 20 tiny ops
            mr = gnp.tile([8, 2 * NCT], F32, tag="mr")
            nc.vector.tensor_copy(out=mr[:, 0:NCT], in_=gs[:, 0:NCT])
            nc.vector.tensor_copy(out=mr[:, NCT:2 * NCT], in_=r0[:, :])
            h_t = []
            with nc.allow_low_precision(reason="bf16 matmul operands"):
                mrc = av_tile("mrcall")
                nc.tensor.matmul(
                    mrc[:, 0:2 * NCT], gbcast[:, :], mr[:, :],
                    start=True, stop=True,
                )
                sca = gnp.tile([128, NCT], F32, tag="scg")
                sha = gnp.tile([128, NCT], F32, tag="shg")
                nc.vector.tensor_mul(
                    out=sca[:, :], in0=mrc[:, NCT:2 * NCT], in1=gamma[:, :]
                )
                nc.vector.tensor_mul(out=sha[:, :], in0=mrc[:, 0:NCT], in1=sca[:, :])
                nc.vector.tensor_sub(out=sha[:, :], in0=beta[:, :], in1=sha[:, :])
                for ct in range(NCT):
                    h_t.append(hp.tile([128, T], BF16, tag="h", name=f"h{ct}"))
                # all first halves before any second half: vT's first matmuls
                # read only columns 0-511 of each h tile, so they unblock
                # after 4 short ScalarE ops instead of 4 full-tile ones
                for hf in range(2):
                    sl = slice(hf * 512, (hf + 1) * 512)
                    for ct in range(NCT):
                        nc.scalar.activation(
                            out=h_t[ct][:, sl], in_=x_t[ct][:, sl],
                            func=AF.Identity, scale=sca[:, ct:ct + 1],
                            bias=sha[:, ct:ct + 1],
                        )

            emit_warmups(9, 11)

            # ------------- pipelined attention helpers -------------
            def emit_qk_mms_half(p, psum, half, nq):
                """One (q|k, t-half) chunk: 4 matmuls into a 1-bank psum."""
                off = 256 * p + 128 * half
                for ct in range(NCT):
                    nc.tensor.matmul(
                        psum[:, 0:512],
                        wq_t[ct][:, off:off + 128],
                        h_t[ct][:, nq * 512:(nq + 1) * 512],
                        start=(ct == 0), stop=(ct == NCT - 1),
                    )

            def emit_qk_copy(p, half, nq, psum, dst):
                """psum->sbuf bf16 with bias on ScalarE, one t-half."""
                with nc.allow_low_precision(reason="bf16 matmul operands"):
                    nc.scalar.activation(
                        out=dst[:, nq * 512:(nq + 1) * 512], in_=psum[:, 0:512],
                        func=AF.Identity,
                        bias=bcols[:, 2 * p + half:2 * p + half + 1],
                    )

            def qk_pair_tile(name):
                return qkp.tile([128, T], BF16, tag="qkpair", name=name)

            # ---------------- v^T (+ per-head Z columns) ----------------
            qp_s = qk_pair_tile("qp0")
            kp_s = qk_pair_tile("kp0")
            vt_t = []
            with nc.allow_low_precision(reason="bf16 av operands"):
                for tt in range(NTT):
                    vps = sc_tile(f"vps{tt}")
                    for seg in ((0, 512), (512, VW)):
                        dst = vps[:, seg[0]:seg[1]]
                        for ct in range(NCT):
                            nc.tensor.matmul(
                                dst,
                                h_t[ct][:, tt * 128:(tt + 1) * 128],
                                wq_t[ct][:, 2 * C + seg[0]:2 * C + seg[1]],
                                start=(ct == 0), stop=False,
                            )
                        nc.tensor.matmul(
                            dst, ones[0:1, 0:128],
                            vbias[0:1, seg[0]:seg[1]],
                            start=False, stop=True,
                        )
                    vt = vtp.tile([128, VW], BF16, tag="vt")
                    nc.vector.tensor_copy(out=vt[:, :], in_=vps[:, 0:VW])
                    vt_t.append(vt)
                    # qk(0) chunks ride the qk ring between vT tiles; their
                    # ScalarE copies hide under the next tile's matmuls
                    if 1 <= tt <= 4:
                        half, nq = (tt - 1) // 2, (tt - 1) % 2
                        qk0ps = qk_tile(f"qk0_{half}_{nq}")
                        emit_qk_mms_half(0, qk0ps, half, nq)
                        emit_qk_copy(0, half, nq, qk0ps,
                                     qp_s if half == 0 else kp_s)

            # fold proj bias into the residual: x += bproj (per-partition)
            # on the otherwise-idle GpSimd engine (SBUF-only op, and the
            # result isn't needed until the tail's residual adds)
            for ct in range(NCT):
                nc.gpsimd.tensor_scalar_add(
                    out=x_t[ct][:, :], in0=x_t[ct][:, :],
                    scalar1=bcols[:, 8 + ct:9 + ct],
                )

            # state carried across pairs
            qk_next_ps = None           # next pair's q or k psum in flight
            qp_next = None
            pts_prev = None             # previous pair's pt tiles
            av_group = [None] * 4       # live av psum tiles by group



            aunbig = aunp.tile([CH + 1, NH, T], F32, tag="aun")
            zallA = zp.tile([4, T], F32, tag="zA")
            zallB1 = zp.tile([2, T], F32, tag="zB1")
            zallB2 = zp.tile([2, T], F32, tag="zB2")
            zlogA = zp.tile([4, T], F32, tag="zlA")
            zscr = zp.tile([4, T], F32, tag="zscr")
            invzA = zp.tile([4, T], F32R, tag="izA")
            invzB1 = zp.tile([2, T], F32R, tag="izB1")
            invzB2 = zp.tile([2, T], F32R, tag="izB2")

            def emit_invz(zall, zlog, invz):
                """invz = 1/Z on the VectorE. (Ln sits in a different ACT
                table set than Exp here, so an ACT-side exp(-ln Z) causes
                ~2.6us of table reloads per batch mid-kernel.)"""
                n = zall.shape[0]
                nc.vector.reciprocal_approx_fast(
                    out=zlog[0:n, :], in_=zall[:, :]
                )
                with nc.allow_low_precision(reason="fp32r matmul operand"):
                    nc.vector.tensor_copy(out=invz[:, :], in_=zlog[0:n, :])

            def emit_av_group_mms(p, g, pts):
                """All 8 accumulating av matmuls of group g=(h,nq) of pair p."""
                hl, nq = g // 2, g % 2
                h_ = 2 * p + hl
                avt = av_tile(f"av{p}_{g}")
                av_group[g] = avt
                for st_ in range(NTT):
                    nc.tensor.matmul(
                        avt[0:CH + 1, :],
                        vt_t[st_][:, h_ * (CH + 1):(h_ + 1) * (CH + 1)],
                        pts[st_][hl][:, nq * 512:(nq + 1) * 512],
                        start=(st_ == 0), stop=(st_ == NTT - 1),
                    )

            def emit_av_group_copy(p, g, engine):
                hl, nq = g // 2, g % 2
                h_ = 2 * p + hl
                avt = av_group[g]
                dst = aunbig[0:CH + 1, h_, nq * 512:(nq + 1) * 512]
                if engine == "act":
                    nc.scalar.activation(
                        out=dst, in_=avt[0:CH + 1, :], func=AF.Identity
                    )
                else:
                    nc.vector.tensor_copy(out=dst, in_=avt[0:CH + 1, :])

            # ---------------- pairs 0..3: scores + exp + av(p-1) ----------
            for p in range(4):
                pts_cur = []
                for st_ in range(NTT):
                    # scores for both heads (row-group packed, 2 concurrent)
                    scA = sc_tile(f"sc{p}_{st_}_0")
                    scB = sc_tile(f"sc{p}_{st_}_1")
                    scps = [scA, scB]
                    for nq in range(2):
                        for hl in (1, 0):
                            base = 64 * hl
                            nc.tensor.matmul(
                                scps[hl][:, nq * 512:(nq + 1) * 512],
                                kp_s[base:base + 64, st_ * 128:(st_ + 1) * 128],
                                qp_s[base:base + 64, nq * 512:(nq + 1) * 512],
                                start=True, stop=True,
                            )
                    # previous pair's av-group copy goes ahead of the exps
                    # so the single av bank frees in time for the next group
                    if p >= 1 and st_ % 2 == 1:
                        g = st_ // 2
                        emit_av_group_copy(
                            p - 1, g, "act" if g % 2 == 0 else "dve"
                        )

                    # next pair's qk copies go ahead of this slot's ACT exp
                    # so the freed qk bank is back in time for the next chunk
                    if p <= 2:
                        pn = p + 1
                        if st_ == 5:
                            qp_next = qk_pair_tile(f"qp{pn}")
                            emit_qk_copy(pn, 0, 0, qk_ps, qp_next)
                        if st_ == 6:
                            emit_qk_copy(pn, 0, 1, qk_ps, qp_next)
                        if st_ == 7:
                            kp_next = qk_pair_tile(f"kp{pn}")
                            emit_qk_copy(pn, 1, 0, qk_ps, kp_next)

                    # exp: odd head Schraudolph on VectorE (its scores ran
                    # first, so its gate clears first), even head on ScalarE
                    with nc.allow_low_precision(reason="bf16 av operands"):
                        ptB = ptBp.tile([128, T], I16, tag="ptB",
                                        name=f"ptB{p}_{st_}")
                        nc.vector.tensor_scalar(
                            out=ptB[:, :], in0=scB[:, :], scalar1=EA,
                            scalar2=EB, op0=ALU.mult, op1=ALU.add,
                        )
                        ptA = ptAp.tile([128, T], BF16, tag="ptA",
                                        name=f"ptA{p}_{st_}")
                        nc.scalar.activation(
                            out=ptA[:, :], in_=scA[:, :], func=AF.Exp
                        )
                    pts_cur.append((ptA[:, :], ptB[:, :].bitcast(BF16)))

                    # filler matmuls in low-density slots: pair 0 has no av
                    # work, pair 3 no qk work; an idle-ish PE re-throttles the
                    # HAM clock gate, so burn the slack on dummy matmuls
                    if p == 0 and st_ < 4:
                        for f_ in range(2):
                            fps = av_tile(f"fill0_{st_}_{f_}")
                            nc.tensor.matmul(
                                fps[:, :], wq_t[0][:, 0:128],
                                h_t[0][:, 0:512], start=True, stop=True,
                            )
                    if p == 3 and st_ % 2 == 1:
                        for f_ in range(2):
                            fps = qk_tile(f"fill3_{st_}_{f_}")
                            nc.tensor.matmul(
                                fps[:, :512], wq_t[0][:, 0:128],
                                h_t[0][:, 0:512], start=True, stop=True,
                            )

                    # av of previous pair: group g = st//2 on even slots
                    if p >= 1 and st_ % 2 == 0:
                        emit_av_group_mms(p - 1, st_ // 2, pts_prev)

                    # pair-3 slack: stage heads 0-3 Z rows (invz runs in
                    # the tail; ACT here would delay this slot's exp gate)
                    if p == 3 and st_ == 0:
                        nc.sync.dma_start(
                            out=zallA[:, :], in_=aunbig[CH:CH + 1, 0:4, :]
                        )

                    # next pair's qk matmul chunks fill slots 4-7
                    if p <= 2:
                        pn = p + 1
                        if st_ == 4:
                            qk_ps = qk_tile(f"q{pn}n0")
                            emit_qk_mms_half(pn, qk_ps, 0, 0)
                        if st_ == 5:
                            qk_ps = qk_tile(f"q{pn}n1")
                            emit_qk_mms_half(pn, qk_ps, 0, 1)
                        if st_ == 6:
                            qk_ps = qk_tile(f"k{pn}n0")
                            emit_qk_mms_half(pn, qk_ps, 1, 0)
                        if st_ == 7:
                            qk_ps = qk_tile(f"k{pn}n1")
                            emit_qk_mms_half(pn, qk_ps, 1, 1)

                if p <= 2:
                    emit_qk_copy(p + 1, 1, 1, qk_ps, kp_next)
                    qp_s, kp_s = qp_next, kp_next
                pts_prev = pts_cur

            # ---------------- tail: av(3) + normalize + projection --------
            a_all = [
                aallp.tile([128, T], BF16, tag="aall", name=f"aall{ct}")
                for ct in range(NCT)
            ]

            def emit_zb_nq(h_, nq):
                """zb[64, 512] psum = one-hot fp32r broadcast matmul of the
                invz row for head h_, one t-half. Heads 6,7 use the av ring
                (the qk ring is held by m2's early projection by then)."""
                mk = av_tile if h_ >= 6 else qk_tile
                zb = mk(f"zb{h_}n{nq}")
                if h_ < 4:
                    st_op, izv = e8, invzA
                else:
                    st_op, izv = e2, (invzB1 if h_ < 6 else invzB2)
                nc.tensor.matmul(
                    zb[0:CH, 0:512],
                    st_op[:, h_ * CH:(h_ + 1) * CH],
                    izv[:, nq * 512:(nq + 1) * 512],
                    start=True, stop=True,
                )
                return zb

            def emit_norm(h_):
                """normalize head h_ into a_all (zb + mul per t-half)."""
                with nc.allow_low_precision(reason="bf16 matmul operands"):
                    if h_ % 2 == 0:
                        for nq in range(2):
                            zb = emit_zb_nq(h_, nq)
                            nc.vector.tensor_mul(
                                out=a_all[h_ // 2][0:CH, nq * 512:(nq + 1) * 512],
                                in0=aunbig[0:CH, h_, nq * 512:(nq + 1) * 512],
                                in1=zb[0:CH, 0:512],
                            )
                    else:
                        # DVE writes the shifted partition range directly
                        # (base 64 is a legal AP base) - no atmp+DMA bounce
                        for nq in range(2):
                            zb = emit_zb_nq(h_, nq)
                            nc.vector.tensor_mul(
                                out=a_all[h_ // 2][CH:2 * CH,
                                                   nq * 512:(nq + 1) * 512],
                                in0=aunbig[0:CH, h_, nq * 512:(nq + 1) * 512],
                                in1=zb[0:CH, 0:512],
                            )

            def emit_proj(m, pps, cks):
                for nq in range(2):
                    dst = pps[:, nq * 512:(nq + 1) * 512]
                    for ck in cks:
                        nc.tensor.matmul(
                            dst,
                            wp_t[ck][:, m * 128:(m + 1) * 128],
                            a_all[ck][:, nq * 512:(nq + 1) * 512],
                            start=(ck == 0), stop=(ck == NCT - 1),
                        )

            def emit_proj_half(m, ph, nq, cks):
                for ck in cks:
                    nc.tensor.matmul(
                        ph[:, 0:512],
                        wp_t[ck][:, m * 128:(m + 1) * 128],
                        a_all[ck][:, nq * 512:(nq + 1) * 512],
                        start=(ck == 0), stop=(ck == NCT - 1),
                    )

            # tail: av(3) + normalize + early projection. invz batches run
            # on ScalarE; with the 1-bank av ring each group's copy directly
            # precedes the next group's matmuls.
            emit_invz(zallA, zlogA, invzA)
            nc.sync.dma_start(
                out=zallB1[:, :], in_=aunbig[CH:CH + 1, 4:6, :]
            )
            emit_invz(zallB1, zlogA[0:2, :], invzB1)
            emit_av_group_mms(3, 0, pts_prev)
            emit_av_group_copy(3, 0, "dve")
            emit_norm(0)
            emit_av_group_mms(3, 1, pts_prev)
            emit_av_group_copy(3, 1, "dve")
            emit_norm(1)
            pps01 = [sc_tile(f"pps{m}") for m in range(2)]
            for m in range(2):
                emit_proj(m, pps01[m], [0])
            emit_av_group_mms(3, 2, pts_prev)
            emit_av_group_copy(3, 2, "dve")
            emit_norm(2)
            emit_norm(3)
            for m in range(2):
                emit_proj(m, pps01[m], [1])
            emit_av_group_mms(3, 3, pts_prev)
            emit_av_group_copy(3, 3, "dve")
            emit_norm(4)
            emit_norm(5)
            for m in range(2):
                emit_proj(m, pps01[m], [2])
            # m2 runs its ck 0-2 accumulation through the qk ring halves
            # while heads 6,7 still normalize (sc slots are held by m0/m1)
            pps2h = []
            for nq in range(2):
                ph = qk_tile(f"pps2n{nq}")
                pps2h.append(ph)
                emit_proj_half(2, ph, nq, [0, 1, 2])

            nc.sync.dma_start(
                out=zallB2[:, :], in_=aunbig[CH:CH + 1, 6:8, :]
            )
            emit_invz(zallB2, zlogA[0:2, :], invzB2)
            emit_norm(6)
            emit_norm(7)
            # m3's ck 0-2 accumulation through the freed av-ring halves,
            # overlapping m0/m1's ck3 work below
            pps3h = []
            for nq in range(2):
                ph = av_tile(f"pps3n{nq}")
                pps3h.append(ph)
                emit_proj_half(3, ph, nq, [0, 1, 2])

            # finish projection + residual + store (per half: PE, DVE
            # and DMA pipeline at half-tile granularity)
            for m in range(2):
                for nq in range(2):
                    sl = slice(nq * 512, (nq + 1) * 512)
                    for ck in [3]:
                        nc.tensor.matmul(
                            pps01[m][:, sl],
                            wp_t[ck][:, m * 128:(m + 1) * 128],
                            a_all[ck][:, sl],
                            start=False, stop=True,
                        )
                    nc.vector.tensor_add(
                        out=x_t[m][:, sl], in0=pps01[m][:, sl],
                        in1=x_t[m][:, sl],
                    )
                    nc.sync.dma_start(
                        out=y_d.ap()[m * 128:(m + 1) * 128, sl],
                        in_=x_t[m][:, sl],
                    )
            for nq in range(2):
                emit_proj_half(2, pps2h[nq], nq, [3])
                nc.vector.tensor_add(
                    out=x_t[2][:, nq * 512:(nq + 1) * 512],
                    in0=pps2h[nq][:, 0:512],
                    in1=x_t[2][:, nq * 512:(nq + 1) * 512],
                )
                nc.sync.dma_start(
                    out=y_d.ap()[2 * 128:3 * 128, nq * 512:(nq + 1) * 512],
                    in_=x_t[2][:, nq * 512:(nq + 1) * 512],
                )
            for nq in range(2):
                emit_proj_half(3, pps3h[nq], nq, [3])
                nc.vector.tensor_add(
                    out=x_t[3][:, nq * 512:(nq + 1) * 512],
                    in0=pps3h[nq][:, 0:512],
                    in1=x_t[3][:, nq * 512:(nq + 1) * 512],
                )
                nc.sync.dma_start(
                    out=y_d.ap()[3 * 128:4 * 128, nq * 512:(nq + 1) * 512],
                    in_=x_t[3][:, nq * 512:(nq + 1) * 512],
                )

    nc.compile()
    return nc


def make_in_maps(x, gn_weight, gn_bias, w_qkv, b_qkv, w_proj, b_proj):
    import ml_dtypes

    x = np.asarray(x, dtype=np.float32)
    w_qkv = np.asarray(w_qkv, dtype=np.float32)
    b_qkv = np.asarray(b_qkv, dtype=np.float32)
    scale = np.float32(SCALE)
    wq = w_qkv.copy()
    bq = b_qkv.copy()
    for hd in range(NH):
        sl = slice(3 * CH * hd, 3 * CH * hd + 2 * CH)  # q,k rows of this head
        wq[sl] *= scale
        bq[sl] *= scale
    # Column order expected by the kernel: per head-pair p the contiguous
    # blocks [q(2p) | q(2p+1) | k(2p) | k(2p+1)] (256 cols each), then all
    # v blocks (with per-head Z columns).
    perm = []
    for p in range(NH // 2):
        for hd in (2 * p, 2 * p + 1):
            perm.extend(range(3 * CH * hd, 3 * CH * hd + CH))          # q
        for hd in (2 * p, 2 * p + 1):
            perm.extend(range(3 * CH * hd + CH, 3 * CH * hd + 2 * CH))  # k
    for hd in range(NH):
        perm.extend(range(3 * CH * hd + 2 * CH, 3 * CH * hd + 3 * CH))  # v
    perm = np.asarray(perm)
    wq = wq[perm]
    bq = bq[perm]
    # v-section gains a zero-weight column per head whose bias is 1.0 (the
    # Z column of v^T); qk section stays 1024 wide.
    wq2 = np.zeros((C, 2 * C + VW), np.float32)
    vb = np.zeros(VW, np.float32)
    wq2[:, 0:2 * C] = wq.T[:, 0:2 * C]
    for hd in range(NH):
        wq2[:, 2 * C + 65 * hd:2 * C + 65 * hd + CH] = \
            wq.T[:, 2 * C + CH * hd:2 * C + CH * (hd + 1)]
        vb[65 * hd:65 * hd + CH] = bq[2 * C + CH * hd:2 * C + CH * (hd + 1)]
        vb[65 * hd + CH] = 1.0
    wqkvT = np.ascontiguousarray(wq2.astype(ml_dtypes.bfloat16))
    wprojT = np.ascontiguousarray(
        np.asarray(w_proj, np.float32).T.astype(ml_dtypes.bfloat16)
    )
    vrow = np.concatenate(
        [vb, np.ones(512, np.float32)]
    ).reshape(1, -1).astype(ml_dtypes.bfloat16)

    gamma = np.asarray(gn_weight, np.float32).reshape(NCT, 128).T
    beta = np.asarray(gn_bias, np.float32).reshape(NCT, 128).T
    gred = np.zeros((128, 8), np.float32)
    gbcast = np.zeros((8, 128), np.float32)
    for c in range(128):
        gred[c, c // 16] = 1.0 / (16.0 * 1024.0)
        gbcast[c // 16, c] = 1.0
    e8 = np.zeros((4, NH * CH), np.float32)
    e2 = np.zeros((2, NH * CH), np.float32)
    for g in range(8):
        e8[g % 4, g * CH:(g + 1) * CH] = 1.0
        e2[g % 2, g * CH:(g + 1) * CH] = 1.0
    cblob = np.ascontiguousarray(np.concatenate([gamma, beta, gred], axis=1))
    # bias columns: per pair p the q col then k col, then bproj columns
    bcols = np.zeros((128, 12), np.float32)
    for p in range(NH // 2):
        bcols[:, 2 * p] = bq[256 * p:256 * p + 128]
        bcols[:, 2 * p + 1] = bq[256 * p + 128:256 * p + 256]
    bcols[:, 8:12] = np.asarray(b_proj, np.float32).reshape(NCT, 128).T

    common = dict(
        wqkvT=wqkvT, wprojT=wprojT, cblob=cblob, bcols=bcols,
        vrow=np.ascontiguousarray(vrow), gbcast=gbcast, e8=e8, e2=e2,
    )
    in_maps = []
    for b in range(B):
        m = dict(common)
        m["x"] = np.ascontiguousarray(x[b].reshape(C, T))
        in_maps.append(m)
    return in_maps


def kernel(x, gn_weight, gn_bias, w_qkv, b_qkv, w_proj, b_proj, _trace=False):
    if "nc" not in _CACHE:
        _CACHE["nc"] = build_kernel()
    nc = _CACHE["nc"]
    in_maps = make_in_maps(x, gn_weight, gn_bias, w_qkv, b_qkv, w_proj, b_proj)
    res = bass_utils.run_bass_kernel_spmd(
        nc, in_maps, core_ids=list(range(B)), trace=_trace
    )
    out = np.stack([r["y"].reshape(C, HH, WW) for r in res.results], axis=0)
    if _trace:
        _CACHE["last_result"] = res
    return out


# revision 52
# speedup vs baseline: 1.0008x; 1.0008x over previous
"""AttentionBlock (GroupNorm + MHSA + proj + residual) on 8 TRN2 NeuronCores.

Sharding: data-parallel over batch (B=8 -> 1 batch element per core), SPMD.

Software-pipelined attention (352us baseline -> ~154us). The
scores->exp->av dependency chain is decoupled by keeping each pair's
exp(S) tiles (pt) in SBUF: pair p's slots run scores(p) + av(p-1) +
qk-projection(p+1) back to back on the PE, with exp (ScalarE exact for
even heads / VectorE Schraudolph bf16 fast-exp for odd heads) hidden
underneath; scores for the two heads of a pair run concurrently on row
groups 0/64 of the PE array. PSUM banks: scores 2x2, av accumulators 2x1,
qk/zb 1x2. Softmax 1/Z is computed as exp(-ln Z) on ScalarE in three
batches so the last heads' normalization hides under the tail's av; the
out-projection is woven into the tail as each normalized a-tile lands.
Other key points vs the original baseline: bf16 weights/activations for
everything except the fp32 GroupNorm + residual, x-first DMA ordering,
PE warm-up + filler matmuls so the HAM clock gate holds 2.4GHz through
low-density phases, GroupNorm rstd via exp(-0.5*ln(var+eps)) so only one
ACT table set is ever loaded, and biases folded into ScalarE psum->sbuf
copies / the residual instead of matmuls.
"""

import sys
import numpy as np

sys.path.insert(0, "/opt/trn_rl_repo")

import concourse.bacc as bacc
import concourse.bass as bass
import concourse.mybir as mybir
import concourse.tile as tile
from concourse import bass_utils

F32 = mybir.dt.float32
F32R = mybir.dt.float32r
BF16 = mybir.dt.bfloat16
I16 = mybir.dt.int16
AF = mybir.ActivationFunctionType
ALU = mybir.AluOpType

B, C, HH, WW = 8, 512, 32, 32
T = HH * WW            # 1024
NH = 8                 # heads
CH = C // NH           # 64 per-head dim
NCT = C // 128         # 4 channel tiles
NTT = T // 128         # 8 seq tiles
SCALE = 1.0 / np.sqrt(np.sqrt(CH))
EPS = 1e-5
VW = NH * (CH + 1)     # 520: v-section width incl per-head Z column

# Schraudolph fast-exp constants, bf16 flavor: i16 = rint(S * EA + EB)
# gives the int16 bit pattern of bf16(~exp(S)). Max multiplicative error
# ~ +-3.5% (centered); cancels partially in softmax.
EA = float((1 << 7) / np.log(2.0))
EB = float(127.0 * (1 << 7) - 0.5 * 0.0860713320559342 * (1 << 7) + 0.5)

_CACHE = {}


def build_kernel(debug=False):
    nc = bacc.Bacc(
        "TRN2", target_bir_lowering=False, debug=debug, num_devices=8
    )

    x_d = nc.dram_tensor("x", (C, T), F32, kind="ExternalInput")
    wqkvT_d = nc.dram_tensor("wqkvT", (C, 2 * C + VW), BF16, kind="ExternalInput")
    wprojT_d = nc.dram_tensor("wprojT", (C, C), BF16, kind="ExternalInput")
    cblob_d = nc.dram_tensor("cblob", (128, 2 * NCT + 8), F32, kind="ExternalInput")
    bcols_d = nc.dram_tensor("bcols", (128, 12), F32, kind="ExternalInput")
    vrow_d = nc.dram_tensor("vrow", (1, VW + 512), BF16, kind="ExternalInput")
    gbcast_d = nc.dram_tensor("gbcast", (8, 128), F32, kind="ExternalInput")
    e8_d = nc.dram_tensor("e8", (4, NH * CH), F32R, kind="ExternalInput")
    e2_d = nc.dram_tensor("e2", (2, NH * CH), F32R, kind="ExternalInput")
    y_d = nc.dram_tensor("y", (C, T), F32, kind="ExternalOutput")

    with tile.TileContext(nc) as tc:
        with (
            tc.tile_pool(name="single", bufs=1) as single,
            tc.tile_pool(name="hp", bufs=NCT) as hp,
            tc.tile_pool(name="vt", bufs=NTT) as vtp,
            tc.tile_pool(name="qk", bufs=4) as qkp,
            tc.tile_pool(name="ptA", bufs=2 * NTT) as ptAp,
            tc.tile_pool(name="ptB", bufs=2 * NTT) as ptBp,
            tc.tile_pool(name="aun", bufs=1) as aunp,
            tc.tile_pool(name="aall", bufs=NCT) as aallp,
            tc.tile_pool(name="zp", bufs=1) as zp,
            tc.tile_pool(name="tmp", bufs=2) as tmpp,
            tc.tile_pool(name="gn", bufs=8) as gnp,
            tc.tile_pool(name="ps", bufs=1, space="PSUM") as pp,
        ):
            # PSUM tags: sc 2x[128,1024] (4 banks), av 2x[128,512] (2),
            # qk 2x[128,512] (2) -> 8 banks exactly. av and qk rings are
            # 2-deep so no group or chunk ever waits on its predecessor's
            # psum->sbuf copy.
            def sc_tile(name):
                return pp.tile([128, T], F32, tag="sc", bufs=2, name=name)

            def av_tile(name):
                return pp.tile([128, 512], F32, tag="av", bufs=2, name=name)

            def qk_tile(name):
                return pp.tile([128, 512], F32, tag="qk", bufs=2, name=name)

            # ---------------- x first, then consts, then weights ----------
            xbig = single.tile([128, NCT, T], F32, tag="xbig")
            xr4 = x_d.ap().rearrange("(c p) t -> p c t", p=128)
            # half-tile granularity: each bn_stats chunk unblocks as soon as
            # its own 512 columns land
            for ct in range(NCT):
                for hf in range(2):
                    sl = slice(hf * 512, (hf + 1) * 512)
                    nc.sync.dma_start(out=xbig[:, ct, sl], in_=xr4[:, ct, sl])
            x_t = [xbig[:, ct, :] for ct in range(NCT)]

            cblob = single.tile([128, 2 * NCT + 8], F32, tag="cblob")
            nc.sync.dma_start(out=cblob[:, :], in_=cblob_d.ap())
            gamma = cblob[:, 0:NCT]
            beta = cblob[:, NCT:2 * NCT]
            gred = cblob[:, 2 * NCT:2 * NCT + 8]
            bcols = single.tile([128, 12], F32, tag="bcols")
            nc.sync.dma_start(out=bcols[:, :], in_=bcols_d.ap())
            gbcast = single.tile([8, 128], F32, tag="gbcast")
            nc.sync.dma_start(out=gbcast[:, :], in_=gbcast_d.ap())
            e8 = single.tile([4, NH * CH], F32R, tag="e8")
            nc.sync.dma_start(out=e8[:, :], in_=e8_d.ap())
            e2 = single.tile([2, NH * CH], F32R, tag="e2")
            nc.sync.dma_start(out=e2[:, :], in_=e2_d.ap())
            vrow = single.tile([1, VW + 512], BF16, tag="vrow")
            nc.sync.dma_start(out=vrow[:, :], in_=vrow_d.ap())
            vbias = vrow[:, 0:VW]
            ones = vrow[:, VW:VW + 512]

            # preload the natural_log_exp ACT table set during DMA wait
            tldm = gnp.tile([1, 1], F32, tag="tld")
            nc.scalar.activation(out=tldm[:, :], in_=cblob[0:1, 0:1], func=AF.Ln)

            wqbig = single.tile([128, NCT, 2 * C + VW], BF16, tag="wqbig")
            wqr = wqkvT_d.ap().rearrange("(c p) t -> p c t", p=128)
            # v-section first (vT is computed first), then qk columns
            nc.sync.dma_start(
                out=wqbig[:, :, 2 * C:2 * C + VW], in_=wqr[:, :, 2 * C:2 * C + VW]
            )
            nc.sync.dma_start(out=wqbig[:, :, 0:2 * C], in_=wqr[:, :, 0:2 * C])
            wq_t = [wqbig[:, ct, :] for ct in range(NCT)]
            wpbig = single.tile([128, NCT, C], BF16, tag="wpbig")
            nc.sync.dma_start(
                out=wpbig[:, :, :],
                in_=wprojT_d.ap().rearrange("(c p) t -> p c t", p=128),
            )
            wp_t = [wpbig[:, ct, :] for ct in range(NCT)]

            # ---------------- PE warm-up (HAM clock gate) -----------------
            # ~5us of fp32 matmuls on x tile 0 flip the PE clock gate to
            # 2.4GHz before the real matmul stream begins.
            def emit_warmups(lo, hi):
                for w in range(lo, hi):
                    wps = av_tile(f"warm{w}")
                    nc.tensor.matmul(
                        wps[:, :], x_t[0][:, 0:128], x_t[0][:, 0:512],
                        start=True, stop=True,
                    )

            emit_warmups(0, 6)

            # ---------------- GroupNorm ----------------
            cs = gnp.tile([128, 2 * NCT], F32, tag="cs")
            for ct in range(NCT):
                xr = x_t[ct][:, :].rearrange("p (n f) -> p n f", f=512)
                st = gnp.tile([128, 2, 6], F32, tag="st")
                for sg in range(2):
                    nc.vector.bn_stats(out=st[:, sg, :], in_=xr[:, sg, :])
                mv = gnp.tile([128, 2], F32, tag="mv")
                nc.vector.bn_aggr(out=mv[:, :], in_=st[:, :, :])
                nc.vector.tensor_copy(out=cs[:, ct:ct + 1], in_=mv[:, 0:1])
                nc.vector.tensor_mul(
                    out=cs[:, NCT + ct:NCT + ct + 1], in0=mv[:, 0:1], in1=mv[:, 0:1]
                )
                nc.vector.tensor_add(
                    out=cs[:, NCT + ct:NCT + ct + 1],
                    in0=cs[:, NCT + ct:NCT + ct + 1],
                    in1=mv[:, 1:2],
                )
            gsp = av_tile("gsp")
            nc.tensor.matmul(
                gsp[0:8, 0:2 * NCT], gred[:, :], cs[:, :], start=True, stop=True
            )
            gs = gnp.tile([8, 2 * NCT], F32, tag="gs")
            nc.vector.tensor_copy(out=gs[:, :], in_=gsp[0:8, 0:2 * NCT])
            emit_warmups(6, 9)
            # var = E[x^2] - mean^2; rstd = exp(-0.5*ln(var+eps))
            t1 = gnp.tile([8, NCT], F32, tag="t1")
            veps = gnp.tile([8, NCT], F32, tag="veps")
            nc.vector.tensor_mul(out=t1[:, :], in0=gs[:, 0:NCT], in1=gs[:, 0:NCT])
            nc.vector.tensor_sub(out=veps[:, :], in0=gs[:, NCT:], in1=t1[:, :])
            nc.vector.tensor_scalar_add(out=veps[:, :], in0=veps[:, :], scalar1=EPS)
            lv = gnp.tile([8, NCT], F32, tag="lv")
            nc.scalar.activation(out=lv[:, :], in_=veps[:, :], func=AF.Ln)
            r0 = gnp.tile([8, NCT], F32, tag="r0")
            nc.scalar.activation(out=r0[:, :], in_=lv[:, :], func=AF.Exp, scale=-0.5)
            # de-interleaved [means | rstds] layout: one broadcast matmul
            # and three batched DVE ops replace 4 matmuls + 20 tiny ops
            mr = gnp.tile([8, 2 * NCT], F32, tag="mr")
            nc.vector.tensor_copy(out=mr[:, 0:NCT], in_=gs[:, 0:NCT])
            nc.vector.tensor_copy(out=mr[:, NCT:2 * NCT], in_=r0[:, :])
            h_t = []
            with nc.allow_low_precision(reason="bf16 matmul operands"):
                mrc = av_tile("mrcall")
                nc.tensor.matmul(
                    mrc[:, 0:2 * NCT], gbcast[:, :], mr[:, :],
                    start=True, stop=True,
                )
                sca = gnp.tile([128, NCT], F32, tag="scg")
                sha = gnp.tile([128, NCT], F32, tag="shg")
                nc.vector.tensor_mul(
                    out=sca[:, :], in0=mrc[:, NCT:2 * NCT], in1=gamma[:, :]
                )
                nc.vector.tensor_mul(out=sha[:, :], in0=mrc[:, 0:NCT], in1=sca[:, :])
                nc.vector.tensor_sub(out=sha[:, :], in0=beta[:, :], in1=sha[:, :])
                for ct in range(NCT):
                    h_t.append(hp.tile([128, T], BF16, tag="h", name=f"h{ct}"))
                # all first halves before any second half: vT's first matmuls
                # read only columns 0-511 of each h tile, so they unblock
                # after 4 short ScalarE ops instead of 4 full-tile ones
                for hf in range(2):
                    sl = slice(hf * 512, (hf + 1) * 512)
                    for ct in range(NCT):
                        nc.scalar.activation(
                            out=h_t[ct][:, sl], in_=x_t[ct][:, sl],
                            func=AF.Identity, scale=sca[:, ct:ct + 1],
                            bias=sha[:, ct:ct + 1],
                        )

            emit_warmups(9, 11)

            # ------------- pipelined attention helpers -------------
            def emit_qk_mms_half(p, psum, half, nq):
                """One (q|k, t-half) chunk: 4 matmuls into a 1-bank psum."""
                off = 256 * p + 128 * half
                for ct in range(NCT):
                    nc.tensor.matmul(
                        psum[:, 0:512],
                        wq_t[ct][:, off:off + 128],
                        h_t[ct][:, nq * 512:(nq + 1) * 512],
                        start=(ct == 0), stop=(ct == NCT - 1),
                    )

            def emit_qk_copy(p, half, nq, psum, dst):
                """psum->sbuf bf16 with bias on ScalarE, one t-half."""
                with nc.allow_low_precision(reason="bf16 matmul operands"):
                    nc.scalar.activation(
                        out=dst[:, nq * 512:(nq + 1) * 512], in_=psum[:, 0:512],
                        func=AF.Identity,
                        bias=bcols[:, 2 * p + half:2 * p + half + 1],
                    )

            def qk_pair_tile(name):
                return qkp.tile([128, T], BF16, tag="qkpair", name=name)

            # ---------------- v^T (+ per-head Z columns) ----------------
            qp_s = qk_pair_tile("qp0")
            kp_s = qk_pair_tile("kp0")
            vt_t = []
            with nc.allow_low_precision(reason="bf16 av operands"):
                for tt in range(NTT):
                    vps = sc_tile(f"vps{tt}")
                    for seg in ((0, 512), (512, VW)):
                        dst = vps[:, seg[0]:seg[1]]
                        for ct in range(NCT):
                            nc.tensor.matmul(
                                dst,
                                h_t[ct][:, tt * 128:(tt + 1) * 128],
                                wq_t[ct][:, 2 * C + seg[0]:2 * C + seg[1]],
                                start=(ct == 0), stop=False,
                            )
                        nc.tensor.matmul(
                            dst, ones[0:1, 0:128],
                            vbias[0:1, seg[0]:seg[1]],
                            start=False, stop=True,
                        )
                    vt = vtp.tile([128, VW], BF16, tag="vt")
                    nc.vector.tensor_copy(out=vt[:, :], in_=vps[:, 0:VW])
                    vt_t.append(vt)
                    # qk(0) chunks ride the qk ring between vT tiles; their
                    # ScalarE copies hide under the next tile's matmuls
                    if 1 <= tt <= 4:
                        half, nq = (tt - 1) // 2, (tt - 1) % 2
                        qk0ps = qk_tile(f"qk0_{half}_{nq}")
                        emit_qk_mms_half(0, qk0ps, half, nq)
                        emit_qk_copy(0, half, nq, qk0ps,
                                     qp_s if half == 0 else kp_s)

            # fold proj bias into the residual: x += bproj (per-partition)
            # on the otherwise-idle GpSimd engine (SBUF-only op, and the
            # result isn't needed until the tail's residual adds)
            for ct in range(NCT):
                nc.gpsimd.tensor_scalar_add(
                    out=x_t[ct][:, :], in0=x_t[ct][:, :],
                    scalar1=bcols[:, 8 + ct:9 + ct],
                )

            # state carried across pairs
            qk_next_ps = None           # next pair's q or k psum in flight
            qp_next = None
            pts_prev = None             # previous pair's pt tiles
            av_group = [None] * 4       # live av psum tiles by group



            aunbig = aunp.tile([CH + 1, NH, T], F32, tag="aun")
            zallA = zp.tile([4, T], F32, tag="zA")
            zallB1 = zp.tile([2, T], F32, tag="zB1")
            zallB2 = zp.tile([2, T], F32, tag="zB2")
            zlogA = zp.tile([4, T], F32, tag="zlA")
            zscr = zp.tile([4, T], F32, tag="zscr")
            invzA = zp.tile([4, T], F32R, tag="izA")
            invzB1 = zp.tile([2, T], F32R, tag="izB1")
            invzB2 = zp.tile([2, T], F32R, tag="izB2")

            def emit_invz(zall, zlog, invz):
                """invz = 1/Z on the VectorE. (Ln sits in a different ACT
                table set than Exp here, so an ACT-side exp(-ln Z) causes
                ~2.6us of table reloads per batch mid-kernel.)"""
                n = zall.shape[0]
                nc.vector.reciprocal_approx_fast(
                    out=zlog[0:n, :], in_=zall[:, :]
                )
                with nc.allow_low_precision(reason="fp32r matmul operand"):
                    nc.vector.tensor_copy(out=invz[:, :], in_=zlog[0:n, :])

            def emit_av_group_mms(p, g, pts):
                """All 8 accumulating av matmuls of group g=(h,nq) of pair p."""
                hl, nq = g // 2, g % 2
                h_ = 2 * p + hl
                avt = av_tile(f"av{p}_{g}")
                av_group[g] = avt
                for st_ in range(NTT):
                    nc.tensor.matmul(
                        avt[0:CH + 1, :],
                        vt_t[st_][:, h_ * (CH + 1):(h_ + 1) * (CH + 1)],
                        pts[st_][hl][:, nq * 512:(nq + 1) * 512],
                        start=(st_ == 0), stop=(st_ == NTT - 1),
                    )

            def emit_av_group_copy(p, g, engine):
                hl, nq = g // 2, g % 2
                h_ = 2 * p + hl
                avt = av_group[g]
                dst = aunbig[0:CH + 1, h_, nq * 512:(nq + 1) * 512]
                if engine == "act":
                    nc.scalar.activation(
                        out=dst, in_=avt[0:CH + 1, :], func=AF.Identity
                    )
                else:
                    nc.vector.tensor_copy(out=dst, in_=avt[0:CH + 1, :])

            # ---------------- pairs 0..3: scores + exp + av(p-1) ----------
            for p in range(4):
                pts_cur = []
                for st_ in range(NTT):
                    # scores for both heads (row-group packed, 2 concurrent)
                    scA = sc_tile(f"sc{p}_{st_}_0")
                    scB = sc_tile(f"sc{p}_{st_}_1")
                    scps = [scA, scB]
                    for nq in range(2):
                        for hl in (1, 0):
                            base = 64 * hl
                            nc.tensor.matmul(
                                scps[hl][:, nq * 512:(nq + 1) * 512],
                                kp_s[base:base + 64, st_ * 128:(st_ + 1) * 128],
                                qp_s[base:base + 64, nq * 512:(nq + 1) * 512],
                                start=True, stop=True,
                            )
                    # previous pair's av-group copy goes ahead of the exps
                    # so the single av bank frees in time for the next group
                    if p >= 1 and st_ % 2 == 1:
                        g = st_ // 2
                        emit_av_group_copy(
                            p - 1, g, "act" if g % 2 == 0 else "dve"
                        )

                    # next pair's qk copies go ahead of this slot's ACT exp
                    # so the freed qk bank is back in time for the next chunk
                    if p <= 2:
                        pn = p + 1
                        if st_ == 5:
                            qp_next = qk_pair_tile(f"qp{pn}")
                            emit_qk_copy(pn, 0, 0, qk_ps, qp_next)
                        if st_ == 6:
                            emit_qk_copy(pn, 0, 1, qk_ps, qp_next)
                        if st_ == 7:
                            kp_next = qk_pair_tile(f"kp{pn}")
                            emit_qk_copy(pn, 1, 0, qk_ps, kp_next)

                    # exp: odd head Schraudolph on VectorE (its scores ran
                    # first, so its gate clears first), even head on ScalarE
                    with nc.allow_low_precision(reason="bf16 av operands"):
                        ptB = ptBp.tile([128, T], I16, tag="ptB",
                                        name=f"ptB{p}_{st_}")
                        nc.vector.tensor_scalar(
                            out=ptB[:, :], in0=scB[:, :], scalar1=EA,
                            scalar2=EB, op0=ALU.mult, op1=ALU.add,
                        )
                        ptA = ptAp.tile([128, T], BF16, tag="ptA",
                                        name=f"ptA{p}_{st_}")
                        nc.scalar.activation(
                            out=ptA[:, :], in_=scA[:, :], func=AF.Exp
                        )
                    pts_cur.append((ptA[:, :], ptB[:, :].bitcast(BF16)))

                    # filler matmuls in low-density slots: pair 0 has no av
                    # work, pair 3 no qk work; an idle-ish PE re-throttles the
                    # HAM clock gate, so burn the slack on dummy matmuls
                    if p == 0 and st_ < 4:
                        for f_ in range(2):
                            fps = av_tile(f"fill0_{st_}_{f_}")
                            nc.tensor.matmul(
                                fps[:, :], wq_t[0][:, 0:128],
                                h_t[0][:, 0:512], start=True, stop=True,
                            )
                    if p == 3 and st_ % 2 == 1:
                        for f_ in range(2):
                            fps = qk_tile(f"fill3_{st_}_{f_}")
                            nc.tensor.matmul(
                                fps[:, :512], wq_t[0][:, 0:128],
                                h_t[0][:, 0:512], start=True, stop=True,
                            )

                    # av of previous pair: group g = st//2 on even slots
                    if p >= 1 and st_ % 2 == 0:
                        emit_av_group_mms(p - 1, st_ // 2, pts_prev)

                    # pair-3 slack: stage heads 0-3 Z rows (invz runs in
                    # the tail; ACT here would delay this slot's exp gate)
                    if p == 3 and st_ == 0:
                        nc.sync.dma_start(
                            out=zallA[:, :], in_=aunbig[CH:CH + 1, 0:4, :]
                        )

                    # next pair's qk matmul chunks fill slots 4-7
                    if p <= 2:
                        pn = p + 1
                        if st_ == 4:
                            qk_ps = qk_tile(f"q{pn}n0")
                            emit_qk_mms_half(pn, qk_ps, 0, 0)
                        if st_ == 5:
                            qk_ps = qk_tile(f"q{pn}n1")
                            emit_qk_mms_half(pn, qk_ps, 0, 1)
                        if st_ == 6:
                            qk_ps = qk_tile(f"k{pn}n0")
                            emit_qk_mms_half(pn, qk_ps, 1, 0)
                        if st_ == 7:
                            qk_ps = qk_tile(f"k{pn}n1")
                            emit_qk_mms_half(pn, qk_ps, 1, 1)

                if p <= 2:
                    emit_qk_copy(p + 1, 1, 1, qk_ps, kp_next)
                    qp_s, kp_s = qp_next, kp_next
                pts_prev = pts_cur

            # ---------------- tail: av(3) + normalize + projection --------
            a_all = [
                aallp.tile([128, T], BF16, tag="aall", name=f"aall{ct}")
                for ct in range(NCT)
            ]

            def emit_zb_nq(h_, nq):
                """zb[64, 512] psum = one-hot fp32r broadcast matmul of the
                invz row for head h_, one t-half. Heads 6,7 use the av ring
                (the qk ring is held by m2's early projection by then)."""
                mk = av_tile if h_ >= 6 else qk_tile
                zb = mk(f"zb{h_}n{nq}")
                if h_ < 4:
                    st_op, izv = e8, invzA
                else:
                    st_op, izv = e2, (invzB1 if h_ < 6 else invzB2)
                nc.tensor.matmul(
                    zb[0:CH, 0:512],
                    st_op[:, h_ * CH:(h_ + 1) * CH],
                    izv[:, nq * 512:(nq + 1) * 512],
                    start=True, stop=True,
                )
                return zb

            def emit_norm(h_):
                """normalize head h_ into a_all (zb + mul per t-half)."""
                with nc.allow_low_precision(reason="bf16 matmul operands"):
                    if h_ % 2 == 0:
                        for nq in range(2):
                            zb = emit_zb_nq(h_, nq)
                            nc.vector.tensor_mul(
                                out=a_all[h_ // 2][0:CH, nq * 512:(nq + 1) * 512],
                                in0=aunbig[0:CH, h_, nq * 512:(nq + 1) * 512],
                                in1=zb[0:CH, 0:512],
                            )
                    else:
                        # DVE writes the shifted partition range directly
                        # (base 64 is a legal AP base) - no atmp+DMA bounce
                        for nq in range(2):
                            zb = emit_zb_nq(h_, nq)
                            nc.vector.tensor_mul(
                                out=a_all[h_ // 2][CH:2 * CH,
                                                   nq * 512:(nq + 1) * 512],
                                in0=aunbig[0:CH, h_, nq * 512:(nq + 1) * 512],
                                in1=zb[0:CH, 0:512],
                            )

            def emit_proj(m, pps, cks):
                for nq in range(2):
                    dst = pps[:, nq * 512:(nq + 1) * 512]
                    for ck in cks:
                        nc.tensor.matmul(
                            dst,
                            wp_t[ck][:, m * 128:(m + 1) * 128],
                            a_all[ck][:, nq * 512:(nq + 1) * 512],
                            start=(ck == 0), stop=(ck == NCT - 1),
                        )

            def emit_proj_half(m, ph, nq, cks):
                for ck in cks:
                    nc.tensor.matmul(
                        ph[:, 0:512],
                        wp_t[ck][:, m * 128:(m + 1) * 128],
                        a_all[ck][:, nq * 512:(nq + 1) * 512],
                        start=(ck == 0), stop=(ck == NCT - 1),
                    )

            # tail: av(3) + normalize + early projection. invz batches run
            # on ScalarE; with the 1-bank av ring each group's copy directly
            # precedes the next group's matmuls.
            emit_invz(zallA, zlogA, invzA)
            nc.sync.dma_start(
                out=zallB1[:, :], in_=aunbig[CH:CH + 1, 4:6, :]
            )
            emit_invz(zallB1, zlogA[0:2, :], invzB1)
            emit_av_group_mms(3, 0, pts_prev)
            emit_av_group_copy(3, 0, "dve")
            emit_norm(0)
            emit_av_group_mms(3, 1, pts_prev)
            emit_av_group_copy(3, 1, "dve")
            emit_norm(1)
            pps01 = [sc_tile(f"pps{m}") for m in range(2)]
            for m in range(2):
                emit_proj(m, pps01[m], [0])
            emit_av_group_mms(3, 2, pts_prev)
            emit_av_group_copy(3, 2, "dve")
            emit_norm(2)
            emit_norm(3)
            for m in range(2):
                emit_proj(m, pps01[m], [1])
            emit_av_group_mms(3, 3, pts_prev)
            emit_av_group_copy(3, 3, "dve")
            emit_norm(4)
            emit_norm(5)
            for m in range(2):
                emit_proj(m, pps01[m], [2])
            # m2 runs its ck 0-2 accumulation through the qk ring halves
            # while heads 6,7 still normalize (sc slots are held by m0/m1)
            pps2h = []
            for nq in range(2):
                ph = qk_tile(f"pps2n{nq}")
                pps2h.append(ph)
                emit_proj_half(2, ph, nq, [0, 1, 2])

            nc.sync.dma_start(
                out=zallB2[:, :], in_=aunbig[CH:CH + 1, 6:8, :]
            )
            emit_invz(zallB2, zlogA[0:2, :], invzB2)
            emit_norm(6)
            emit_norm(7)
            # m3's ck 0-2 accumulation through the freed av-ring halves,
            # overlapping m0/m1's ck3 work below
            pps3h = []
            for nq in range(2):
                ph = av_tile(f"pps3n{nq}")
                pps3h.append(ph)
                emit_proj_half(3, ph, nq, [0, 1, 2])

            # finish projection + residual + store (per half: PE, DVE
            # and DMA pipeline at half-tile granularity)
            for m in range(2):
                for nq in range(2):
                    sl = slice(nq * 512, (nq + 1) * 512)
                    for ck in [3]:
                        nc.tensor.matmul(
                            pps01[m][:, sl],
                            wp_t[ck][:, m * 128:(m + 1) * 128],
                            a_all[ck][:, sl],
                            start=False, stop=True,
                        )
                    nc.vector.tensor_add(
                        out=x_t[m][:, sl], in0=pps01[m][:, sl],
                        in1=x_t[m][:, sl],
                    )
                    nc.sync.dma_start(
                        out=y_d.ap()[m * 128:(m + 1) * 128, sl],
                        in_=x_t[m][:, sl],
                    )
            for nq in range(2):
                emit_proj_half(2, pps2h[nq], nq, [3])
                nc.vector.tensor_add(
                    out=x_t[2][:, nq * 512:(nq + 1) * 512],
                    in0=pps2h[nq][:, 0:512],
                    in1=x_t[2][:, nq * 512:(nq + 1) * 512],
                )
                nc.sync.dma_start(
                    out=y_d.ap()[2 * 128:3 * 128, nq * 512:(nq + 1) * 512],
                    in_=x_t[2][:, nq * 512:(nq + 1) * 512],
                )
            for nq in range(2):
                emit_proj_half(3, pps3h[nq], nq, [3])
                nc.vector.tensor_add(
                    out=x_t[3][:, nq * 512:(nq + 1) * 512],
                    in0=pps3h[nq][:, 0:512],
                    in1=x_t[3][:, nq * 512:(nq + 1) * 512],
                )
                nc.sync.dma_start(
                    out=y_d.ap()[3 * 128:4 * 128, nq * 512:(nq + 1) * 512],
                    in_=x_t[3][:, nq * 512:(nq + 1) * 512],
                )

    nc.compile()
    return nc


def make_in_maps(x, gn_weight, gn_bias, w_qkv, b_qkv, w_proj, b_proj):
    import ml_dtypes

    x = np.asarray(x, dtype=np.float32)
    w_qkv = np.asarray(w_qkv, dtype=np.float32)
    b_qkv = np.asarray(b_qkv, dtype=np.float32)
    scale = np.float32(SCALE)
    wq = w_qkv.copy()
    bq = b_qkv.copy()
    for hd in range(NH):
        sl = slice(3 * CH * hd, 3 * CH * hd + 2 * CH)  # q,k rows of this head
        wq[sl] *= scale
        bq[sl] *= scale
    # Column order expected by the kernel: per head-pair p the contiguous
    # blocks [q(2p) | q(2p+1) | k(2p) | k(2p+1)] (256 cols each), then all
    # v blocks (with per-head Z columns).
    perm = []
    for p in range(NH // 2):
        for hd in (2 * p, 2 * p + 1):
            perm.extend(range(3 * CH * hd, 3 * CH * hd + CH))          # q
        for hd in (2 * p, 2 * p + 1):
            perm.extend(range(3 * CH * hd + CH, 3 * CH * hd + 2 * CH))  # k
    for hd in range(NH):
        perm.extend(range(3 * CH * hd + 2 * CH, 3 * CH * hd + 3 * CH))  # v
    perm = np.asarray(perm)
    wq = wq[perm]
    bq = bq[perm]
    # v-section gains a zero-weight column per head whose bias is 1.0 (the
    # Z column of v^T); qk section stays 1024 wide.
    wq2 = np.zeros((C, 2 * C + VW), np.float32)
    vb = np.zeros(VW, np.float32)
    wq2[:, 0:2 * C] = wq.T[:, 0:2 * C]
    for hd in range(NH):
        wq2[:, 2 * C + 65 * hd:2 * C + 65 * hd + CH] = \
            wq.T[:, 2 * C + CH * hd:2 * C + CH * (hd + 1)]
        vb[65 * hd:65 * hd + CH] = bq[2 * C + CH * hd:2 * C + CH * (hd + 1)]
        vb[65 * hd + CH] = 1.0
    wqkvT = np.ascontiguousarray(wq2.astype(ml_dtypes.bfloat16))
    wprojT = np.ascontiguousarray(
        np.asarray(w_proj, np.float32).T.astype(ml_dtypes.bfloat16)
    )
    vrow = np.concatenate(
        [vb, np.ones(512, np.float32)]
    ).reshape(1, -1).astype(ml_dtypes.bfloat16)

    gamma = np.asarray(gn_weight, np.float32).reshape(NCT, 128).T
    beta = np.asarray(gn_bias, np.float32).reshape(NCT, 128).T
    gred = np.zeros((128, 8), np.float32)
    gbcast = np.zeros((8, 128), np.float32)
    for c in range(128):
        gred[c, c // 16] = 1.0 / 16.0
        gbcast[c // 16, c] = 1.0
    e8 = np.zeros((4, NH * CH), np.float32)
    e2 = np.zeros((2, NH * CH), np.float32)
    for g in range(8):
        e8[g % 4, g * CH:(g + 1) * CH] = 1.0
        e2[g % 2, g * CH:(g + 1) * CH] = 1.0
    cblob = np.ascontiguousarray(np.concatenate([gamma, beta, gred], axis=1))
    # bias columns: per pair p the q col then k col, then bproj columns
    bcols = np.zeros((128, 12), np.float32)
    for p in range(NH // 2):
        bcols[:, 2 * p] = bq[256 * p:256 * p + 128]
        bcols[:, 2 * p + 1] = bq[256 * p + 128:256 * p + 256]
    bcols[:, 8:12] = np.asarray(b_proj, np.float32).reshape(NCT, 128).T

    common = dict(
        wqkvT=wqkvT, wprojT=wprojT, cblob=cblob, bcols=bcols,
        vrow=np.ascontiguousarray(vrow), gbcast=gbcast, e8=e8, e2=e2,
    )
    in_maps = []
    for b in range(B):
        m = dict(common)
        m["x"] = np.ascontiguousarray(x[b].reshape(C, T))
        in_maps.append(m)
    return in_maps


def kernel(x, gn_weight, gn_bias, w_qkv, b_qkv, w_proj, b_proj, _trace=False):
    if "nc" not in _CACHE:
        _CACHE["nc"] = build_kernel()
    nc = _CACHE["nc"]
    in_maps = make_in_maps(x, gn_weight, gn_bias, w_qkv, b_qkv, w_proj, b_proj)
    res = bass_utils.run_bass_kernel_spmd(
        nc, in_maps, core_ids=list(range(B)), trace=_trace
    )
    out = np.stack([r["y"].reshape(C, HH, WW) for r in res.results], axis=0)
    if _trace:
        _CACHE["last_result"] = res
    return out
